# revision 7
# baseline (speedup 1.0000x reference)
"""Trainium2 Bass kernel: batched single-channel 3x3 valid conv, 16 output channels.

reference: x [32, 512, 512] f32, kernels [16, 3, 3] f32
           -> out [32, 16, 510, 510] f32  (cross-correlation, VALID, stride 1)

Strategy (memory-regime problem: output is 532 MB, input 33 MB):
  - Data-parallel: 4 images per core across 8 cores; kernels replicated.
  - Inputs are cast to bf16 on the host: the PE runs bf16 matmuls at
    1 cycle/row (fp32 is 4x slower) and input HBM reads halve. PSUM
    accumulation stays fp32; measured rel err ~1e-3 vs the 2e-2 gate.
  - 30-row output blocks (510 = 17 x 30): per block one PE matmul per
    channel-group with contraction K = 3 column-shifts x 32 input rows = 96
    against a host-precomputed banded lhsT [96, 120] (M = 4 channels x 30
    rows).
  - DMA batching (the HW bottleneck is per-dma_start overhead, not bytes):
    ONE input DMA per half-image (9 blocks) whose 4-dim source AP reads the
    3 overlapping column windows for all blocks; ONE output DMA per
    (channel-group, half-image) via a 4-dim DRAM AP covering 4 channel
    planes. 8 input + 32 output DMA calls per core instead of 68 + 128.
  - Input loads go through gpsimd (SWDGE) so both HWDGE rings (sync=SP,
    scalar=ACT) are dedicated to the large output writes, alternating.
  - PSUM [120, 510] tiles are copied (ScalarE/VectorE alternating) into
    per-channel-group staging tiles [120, 9*510] f32.
"""

import numpy as np
import ml_dtypes

import concourse.bass as bass
import concourse.mybir as mybir
import concourse.tile as tile
from concourse import bacc
from concourse.bass_utils import run_bass_kernel_spmd

N_CORES = 8
B, H, W = 32, 512, 512
KN, KS = 16, 3
OH, OW = H - KS + 1, W - KS + 1  # 510, 510
B_LOC = B // N_CORES  # 4

ROWS = 30                # output rows per block (510 = 17 * 30)
IN_ROWS = ROWS + KS - 1  # 32 input rows per block
KDIM = KS * IN_ROWS      # 96 contraction
NBLK = OH // ROWS        # 17
KG = 4                   # channels per matmul group
N_GROUPS = KN // KG      # 4
M = KG * ROWS            # 120 psum partitions
HALVES = [(0, 9), (9, 8)]  # (first block, n blocks) per flush

F32 = mybir.dt.float32

IN_DTS = {
    "bf16": (mybir.dt.bfloat16, ml_dtypes.bfloat16),
    "f32r": (mybir.dt.float32r, np.float32),
    "f32": (mybir.dt.float32, np.float32),
}


def _build_nc(
    in_dt="bf16",
    in_ring="gpsimd",
    out_rings=("sync", "scalar"),
    img_flush=True,
    in_batch=True,
    in_bufs=2,
):
    """DMA APs are limited to 3 dims, so output flushes are per (channel,
    span-of-blocks): DRAM dims (y:30, blk, x:510). img_flush=True flushes a
    whole image per channel (64 output DMAs/core, stage bufs=1); False
    flushes half-images (128 DMAs/core, stage bufs=2).

    in_batch=True loads the input with 3 DMAs per flush-span (one per
    column-shift dx: DRAM dims (y':32, blk:hn, x:510) into partition slice
    [32dx:32dx+32] of a [96, hn*510] tile) instead of one [96, 510] DMA
    per block."""
    bir_dt, _ = IN_DTS[in_dt]
    flushes = [(0, NBLK)] if img_flush else HALVES
    stage_bufs = 1 if img_flush else 2
    nc = bacc.Bacc("TRN2", target_bir_lowering=False, debug=False)
    x_t = nc.dram_tensor("x", [B_LOC, H, W], bir_dt, kind="ExternalInput")
    w_t = nc.dram_tensor("w", [KDIM, N_GROUPS * M], bir_dt, kind="ExternalInput")
    out_t = nc.dram_tensor("out", [B_LOC, KN, OH, OW], F32, kind="ExternalOutput")

    with tile.TileContext(nc) as tc:
        with (
            tc.tile_pool(name="wpool", bufs=1) as wpool,
            tc.tile_pool(name="inpool", bufs=in_bufs if in_batch else 4) as inpool,
            tc.tile_pool(name="psum", bufs=8, space="PSUM") as psum_pool,
            tc.tile_pool(name="stage", bufs=stage_bufs) as stage_pool,
        ):
            wt = wpool.tile([KDIM, N_GROUPS * M], bir_dt)
            nc.sync.dma_start(out=wt[:, :], in_=w_t[:, :])
            cp = 0
            flush = 0
            for b in range(B_LOC):
                for h0, hn in flushes:
                    bigs = [
                        stage_pool.tile(
                            [M, hn * OW], F32, name=f"big{g}", tag=f"big{g}"
                        )
                        for g in range(N_GROUPS)
                    ]
                    src = x_t.ap()[b]  # [H, W]
                    if in_batch:
                        span = inpool.tile([KDIM, hn * OW], bir_dt, tag="in")
                        for dx in range(KS):
                            # DRAM (y':32, blk:hn, x:510) -> partitions
                            # [32dx, 32dx+32), free (blk, x)
                            getattr(nc, in_ring).dma_start(
                                out=span[
                                    dx * IN_ROWS : (dx + 1) * IN_ROWS, :
                                ],
                                in_=bass.AP(
                                    src.tensor,
                                    src.offset + h0 * ROWS * W + dx,
                                    [[W, IN_ROWS], [ROWS * W, hn], [1, OW]],
                                ),
                            )
                    for j in range(hn):
                        if in_batch:
                            base = span[:, j * OW : (j + 1) * OW]
                        else:
                            r = (h0 + j) * ROWS
                            tile_ = inpool.tile([KDIM, OW], bir_dt, tag="in")
                            getattr(nc, in_ring).dma_start(
                                out=tile_[:, :],
                                in_=bass.AP(
                                    src.tensor,
                                    src.offset + r * W,
                                    [[1, KS], [W, IN_ROWS], [1, OW]],
                                ),
                            )
                            base = tile_[:, :]
                        for g in range(N_GROUPS):
                            ps = psum_pool.tile([M, OW], F32)
                            nc.tensor.matmul(
                                ps[:, :],
                                lhsT=wt[:, g * M : (g + 1) * M],
                                rhs=base,
                                start=True,
                                stop=True,
                            )
                            dst = bigs[g][:, j * OW : (j + 1) * OW]
                            if cp % 2 == 0:
                                nc.scalar.copy(out=dst, in_=ps[:, :])
                            else:
                                nc.vector.tensor_copy(out=dst, in_=ps[:, :])
                            cp += 1
                    for g in range(N_GROUPS):
                        for k in range(KG):
                            # DRAM dims (y:30, blk:hn, x:510) match SBUF
                            # (p=y, f=(blk, x)). Stage partition layout is
                            # interleaved (p = y*KG + k) so each channel's
                            # stripe spans partitions 0-119 and its DMA
                            # sprays across ~all 16 SDMA engines instead of
                            # one 30-partition band (half the engines).
                            view = out_t[
                                b,
                                g * KG + k,
                                h0 * ROWS : (h0 + hn) * ROWS,
                                :,
                            ].rearrange("(blk y) x -> y blk x", y=ROWS)
                            ring = out_rings[flush % len(out_rings)]
                            flush += 1
                            getattr(nc, ring).dma_start(
                                out=view,
                                in_=bigs[g][k::KG, 0 : hn * OW],
                            )
    nc.finalize()
    return nc


def _pack_weights(kernels: np.ndarray) -> np.ndarray:
    """lhsT pack (channel-interleaved M): w[dx*IN_ROWS + y + dy, g*M + y*KG + k]
    = kernels[g*KG+k, dy, dx].

    psum[y*KG + k, n] = sum_{dx, y'} lhsT[dx*IN_ROWS + y', y*KG + k]
                                     * x[r + y', n + dx]
                      = sum_{dy, dx} kernels[g*KG+k, dy, dx] * x[r + y + dy, n + dx]
    """
    w = np.zeros((KDIM, N_GROUPS * M), np.float32)
    y = np.arange(ROWS)
    for g in range(N_GROUPS):
        for dx in range(KS):
            for k in range(KG):
                for dy in range(KS):
                    w[dx * IN_ROWS + y + dy, g * M + y * KG + k] = kernels[
                        g * KG + k, dy, dx
                    ]
    return w


def make_in_maps(x, kernels, in_dt="bf16"):
    _, np_dt = IN_DTS[in_dt]
    x = np.ascontiguousarray(np.asarray(x, dtype=np.float32)).astype(np_dt)
    wp = _pack_weights(np.asarray(kernels, dtype=np.float32)).astype(np_dt)
    return [
        {"x": x[c * B_LOC : (c + 1) * B_LOC], "w": wp} for c in range(N_CORES)
    ]


def run(x, kernels, trace=False, in_dt="bf16", **build_kwargs):
    assert np.asarray(x).shape == (B, H, W)
    assert np.asarray(kernels).shape == (KN, KS, KS)
    nc = _build_nc(in_dt=in_dt, **build_kwargs)
    in_maps = make_in_maps(x, kernels, in_dt=in_dt)
    res = run_bass_kernel_spmd(
        nc, in_maps, core_ids=list(range(N_CORES)), trace=trace
    )
    out = np.concatenate([res.results[c]["out"] for c in range(N_CORES)], axis=0)
    return out, res


def kernel(x, kernels):
    out, _ = run(x, kernels)
    return out


# revision 12
# speedup vs baseline: 1.3270x; 1.3270x over previous
"""Trainium2 Bass kernel: batched single-channel 3x3 valid conv, 16 output channels.

reference: x [32, 512, 512] f32, kernels [16, 3, 3] f32
           -> out [32, 16, 510, 510] f32  (cross-correlation, VALID, stride 1)

Strategy (memory-regime problem: output is 532 MB, input 33 MB):
  - Data-parallel: 4 images per core across 8 cores; kernels replicated.
  - Inputs are cast to bf16 on the host: the PE runs bf16 matmuls at
    1 cycle/row (fp32 is 4x slower) and input HBM reads halve. PSUM
    accumulation stays fp32; measured rel err ~1e-3 vs the 2e-2 gate.
  - 30-row output blocks (510 = 17 x 30): per block one PE matmul per
    channel-group with contraction K = 3 column-shifts x 32 input rows = 96
    against a host-precomputed banded lhsT [96, 120] (M = 4 channels x 30
    rows).
  - DMA batching (the HW bottleneck is per-dma_start overhead, not bytes):
    ONE input DMA per half-image (9 blocks) whose 4-dim source AP reads the
    3 overlapping column windows for all blocks; ONE output DMA per
    (channel-group, half-image) via a 4-dim DRAM AP covering 4 channel
    planes. 8 input + 32 output DMA calls per core instead of 68 + 128.
  - Input loads go through gpsimd (SWDGE) so both HWDGE rings (sync=SP,
    scalar=ACT) are dedicated to the large output writes, alternating.
  - PSUM [120, 510] tiles are copied (ScalarE/VectorE alternating) into
    per-channel-group staging tiles [120, 9*510] f32.
"""

import numpy as np
import ml_dtypes

import concourse.bass as bass
import concourse.mybir as mybir
import concourse.tile as tile
from concourse import bacc
from concourse.bass_utils import run_bass_kernel_spmd

N_CORES = 8
B, H, W = 32, 512, 512
KN, KS = 16, 3
OH, OW = H - KS + 1, W - KS + 1  # 510, 510
B_LOC = B // N_CORES  # 4

ROWS = 30                # output rows per block (510 = 17 * 30)
IN_ROWS = ROWS + KS - 1  # 32 input rows per block
KDIM = KS * IN_ROWS      # 96 contraction
NBLK = OH // ROWS        # 17
KG = 4                   # channels per matmul group
N_GROUPS = KN // KG      # 4
M = KG * ROWS            # 120 psum partitions
HALVES = [(0, 9), (9, 8)]  # (first block, n blocks) per flush

F32 = mybir.dt.float32

IN_DTS = {
    "bf16": (mybir.dt.bfloat16, ml_dtypes.bfloat16),
    "f32r": (mybir.dt.float32r, np.float32),
    "f32": (mybir.dt.float32, np.float32),
}


def _build_nc(
    in_dt="bf16",
    in_ring="scalar",
    out_rings=("sync",),
    img_flush=True,
    in_batch=True,
    in_bufs=2,
    split_copy=True,
    out_sp=True,
):
    """DMA APs are limited to 3 dims, so output flushes are per (channel,
    span-of-blocks): DRAM dims (y:30, blk, x:510). img_flush=True flushes a
    whole image per channel (64 output DMAs/core, stage bufs=1); False
    flushes half-images (128 DMAs/core, stage bufs=2).

    in_batch=True loads the input with 3 DMAs per flush-span (one per
    column-shift dx: DRAM dims (y':32, blk:hn, x:510) into partition slice
    [32dx:32dx+32] of a [96, hn*510] tile) instead of one [96, 510] DMA
    per block."""
    bir_dt, _ = IN_DTS[in_dt]
    flushes = [(0, NBLK)] if img_flush else HALVES
    stage_bufs = 1 if img_flush else 2
    nc = bacc.Bacc("TRN2", target_bir_lowering=False, debug=False)
    x_t = nc.dram_tensor("x", [B_LOC, H, W], bir_dt, kind="ExternalInput")
    w_t = nc.dram_tensor("w", [KDIM, N_GROUPS * M], bir_dt, kind="ExternalInput")
    out_t = nc.dram_tensor("out", [B_LOC, KN, OH, OW], F32, kind="ExternalOutput")

    with tile.TileContext(nc) as tc:
        with (
            tc.tile_pool(name="wpool", bufs=1) as wpool,
            tc.tile_pool(name="inpool", bufs=in_bufs if in_batch else 4) as inpool,
            tc.tile_pool(name="psum", bufs=8, space="PSUM") as psum_pool,
            tc.tile_pool(name="stage", bufs=stage_bufs) as stage_pool,
        ):
            wt = wpool.tile([KDIM, N_GROUPS * M], bir_dt)
            nc.sync.dma_start(out=wt[:, :], in_=w_t[:, :])
            cp = 0
            flush = 0
            for b in range(B_LOC):
                for h0, hn in flushes:
                    bigs = [
                        stage_pool.tile(
                            [M, hn * OW], F32, name=f"big{g}", tag=f"big{g}"
                        )
                        for g in range(N_GROUPS)
                    ]
                    src = x_t.ap()[b]  # [H, W]
                    if in_batch:
                        span = inpool.tile([KDIM, hn * OW], bir_dt, tag="in")
                        for dx in range(KS):
                            # DRAM (y':32, blk:hn, x:510) -> partitions
                            # [32dx, 32dx+32), free (blk, x)
                            getattr(nc, in_ring).dma_start(
                                out=span[
                                    dx * IN_ROWS : (dx + 1) * IN_ROWS, :
                                ],
                                in_=bass.AP(
                                    src.tensor,
                                    src.offset + h0 * ROWS * W + dx,
                                    [[W, IN_ROWS], [ROWS * W, hn], [1, OW]],
                                ),
                            )
                    for j in range(hn):
                        if in_batch:
                            base = span[:, j * OW : (j + 1) * OW]
                        else:
                            r = (h0 + j) * ROWS
                            tile_ = inpool.tile([KDIM, OW], bir_dt, tag="in")
                            getattr(nc, in_ring).dma_start(
                                out=tile_[:, :],
                                in_=bass.AP(
                                    src.tensor,
                                    src.offset + r * W,
                                    [[1, KS], [W, IN_ROWS], [1, OW]],
                                ),
                            )
                            base = tile_[:, :]
                        for g in range(N_GROUPS):
                            ps = psum_pool.tile([M, OW], F32)
                            nc.tensor.matmul(
                                ps[:, :],
                                lhsT=wt[:, g * M : (g + 1) * M],
                                rhs=base,
                                start=True,
                                stop=True,
                            )
                            dst = bigs[g][:, j * OW : (j + 1) * OW]
                            if split_copy:
                                # halve the free dim across both engines so
                                # the copy wall time is ~half (cost is
                                # free-dim-bound, partition-independent)
                                hw_ = OW // 2
                                nc.scalar.copy(
                                    out=bigs[g][:, j * OW : j * OW + hw_],
                                    in_=ps[:, 0:hw_],
                                )
                                nc.vector.tensor_copy(
                                    out=bigs[g][:, j * OW + hw_ : (j + 1) * OW],
                                    in_=ps[:, hw_:OW],
                                )
                            elif cp % 2 == 0:
                                nc.scalar.copy(out=dst, in_=ps[:, :])
                            else:
                                nc.vector.tensor_copy(out=dst, in_=ps[:, :])
                            cp += 1
                    for g in range(N_GROUPS):
                        for k in range(KG):
                            # DRAM dims (y:30, blk:hn, x:510) match SBUF
                            # (p=y, f=(blk, x))
                            view = out_t[
                                b,
                                g * KG + k,
                                h0 * ROWS : (h0 + hn) * ROWS,
                                :,
                            ].rearrange("(blk y) x -> y blk x", y=ROWS)
                            ring = out_rings[flush % len(out_rings)]
                            flush += 1
                            getattr(nc, ring).dma_start(
                                out=view,
                                in_=bigs[g][k * ROWS : (k + 1) * ROWS, 0 : hn * OW],
                                single_packet=out_sp,
                            )
    nc.finalize()
    return nc


def _pack_weights(kernels: np.ndarray) -> np.ndarray:
    """lhsT pack: w[dx*IN_ROWS + y + dy, g*M + k*ROWS + y] = kernels[g*KG+k, dy, dx].

    psum[k*ROWS + y, n] = sum_{dx, y'} lhsT[dx*IN_ROWS + y', k*ROWS + y]
                                       * x[r + y', n + dx]
                        = sum_{dy, dx} kernels[g*KG+k, dy, dx] * x[r + y + dy, n + dx]
    """
    w = np.zeros((KDIM, N_GROUPS * M), np.float32)
    y = np.arange(ROWS)
    for g in range(N_GROUPS):
        for dx in range(KS):
            for k in range(KG):
                for dy in range(KS):
                    w[dx * IN_ROWS + y + dy, g * M + k * ROWS + y] = kernels[
                        g * KG + k, dy, dx
                    ]
    return w


def make_in_maps(x, kernels, in_dt="bf16"):
    _, np_dt = IN_DTS[in_dt]
    x = np.ascontiguousarray(np.asarray(x, dtype=np.float32)).astype(np_dt)
    wp = _pack_weights(np.asarray(kernels, dtype=np.float32)).astype(np_dt)
    return [
        {"x": x[c * B_LOC : (c + 1) * B_LOC], "w": wp} for c in range(N_CORES)
    ]


def run(x, kernels, trace=False, in_dt="bf16", **build_kwargs):
    assert np.asarray(x).shape == (B, H, W)
    assert np.asarray(kernels).shape == (KN, KS, KS)
    nc = _build_nc(in_dt=in_dt, **build_kwargs)
    in_maps = make_in_maps(x, kernels, in_dt=in_dt)
    res = run_bass_kernel_spmd(
        nc, in_maps, core_ids=list(range(N_CORES)), trace=trace
    )
    out = np.concatenate([res.results[c]["out"] for c in range(N_CORES)], axis=0)
    return out, res


def kernel(x, kernels):
    out, _ = run(x, kernels)
    return out
